# revision 1
# baseline (speedup 1.0000x reference)
"""Trainium2 Bass kernel for nn_AZConv2d (fuzzy-rule hyperbolic-geometry message passing).

Self-contained: hardcodes shapes B=8,C=64,H=W=128,R=4,Cout=64; shards batch over 8 cores.

Per-core algorithm (image b, all in "transposed land" [col gamma on partitions, row rho in free]):
  1. PE proj per row: lhsT = x-slab [65,128] (x + ones row), rhs = wproj [65, 272]
     -> PSUM [128gamma, 272] = [gate(4) | theta(4) | rawb(4) | rawh(4) | z (256 = 4r x 64o)]
     where z weights fold pw_w @ value_w (and pw_b/4, gate_b, geom_b via the ones row).
  2. Fields (DVE/ACT) in [gamma_p, (rule, rho)_f] planes, pair-symmetric kern:
     kern[r,-s] = shift(kern[r,s]) -> only 4 tap-pairs computed.
     Half-angle identities remove atan2/cos/sin; e^{softplus x} = 1+e^x.
  3. All cross-column (partition-dim) shifts via PE matmuls with constant shift
     matrices (exact zero pad at edges; partition-offset APs are illegal on DVE).
  4. Stencil (DVE scalar_tensor_tensor FMA): out^T[g, o] += w[r,s][g] * zsh[g, rho+dy, r, o]
  5. DMA out^T slabs -> DRAM [l, o]; host transposes to [o, H, W].
"""
import numpy as np
from contextlib import ExitStack

import concourse.bass as bass
import concourse.tile as tile
from concourse import mybir
from concourse.bass_utils import run_bass_kernel_spmd

F32 = mybir.dt.float32
BF16 = mybir.dt.bfloat16
AF = mybir.ActivationFunctionType
OP = mybir.AluOpType

B, C, H, W, R, Cout = 8, 64, 128, 128, 4, 64
L = H * W
NCORE = 8
NFEAT = 16 + R * Cout  # 272
STRIP = 32
NSTRIP = H // STRIP
SW = STRIP + 2          # field window rows per strip
ZW = STRIP + 2          # z shifted-window rows per strip
PAIRS = [(0, 1), (1, -1), (1, 0), (1, 1)]
GP_FRAC_NUM, GP_FRAC_DEN = 1, 3   # fraction of stencil ops on GpSimd
HALF_PI = float(np.pi / 2)

_CACHE = {}


def split_multiwaits(nc):
    """This walrus accepts ONE sync wait per instruction: split extras into
    same-engine NoOps inserted just before the instruction."""
    n = 0
    for bb in nc.main_func.blocks:
        out = []
        for ins in bb.instructions:
            si = ins.sync_info
            if si is not None and len(si.on_wait) > 1:
                waits = list(si.on_wait)
                for w in waits[:-1]:
                    n += 1
                    nop = mybir.InstNoOp(name=f"WSPLIT-{n}")
                    nop.engine = ins.engine
                    nop.sync_info = mybir.SyncInfo(on_wait=[w], on_update=[])
                    out.append(nop)
                ins.sync_info = mybir.SyncInfo(on_wait=[waits[-1]],
                                               on_update=list(si.on_update))
            out.append(ins)
        bb.instructions[:] = out
    return n


def build_program(z_dtype=BF16, debug=False):
    nc = bass.Bass()
    x_d = nc.dram_tensor("x", [C + 1, L], F32, kind="ExternalInput")
    wproj_d = nc.dram_tensor("wproj", [C + 1, NFEAT], F32, kind="ExternalInput")
    smat_d = nc.dram_tensor("smat", [128, 256], F32, kind="ExternalInput")
    out_d = nc.dram_tensor("out", [L, Cout], F32, kind="ExternalOutput")
    dbg = None
    if debug:
        dbg = {
            "dbg_mu": nc.dram_tensor("dbg_mu", [128, R, H + 2], F32,
                                     kind="ExternalOutput")[:],
            "dbg_z": nc.dram_tensor("dbg_z", [128, ZW, R * Cout], F32,
                                    kind="ExternalOutput")[:],
            "dbg_w": nc.dram_tensor("dbg_w", [128, 9, R, STRIP], F32,
                                    kind="ExternalOutput")[:],
            "dbg_gq": nc.dram_tensor("dbg_gq", [128, 16, H + 2], F32,
                                     kind="ExternalOutput")[:],
            "dbg_kern": nc.dram_tensor("dbg_kern", [128, 4, R, STRIP], F32,
                                       kind="ExternalOutput")[:],
            "dbg_den": nc.dram_tensor("dbg_den", [128, R, STRIP], F32,
                                      kind="ExternalOutput")[:],
            "dbg_q": nc.dram_tensor("dbg_q", [128, 4, R, STRIP], F32,
                                    kind="ExternalOutput")[:],
            "dbg_E": nc.dram_tensor("dbg_E", [128, 4, R, STRIP], F32,
                                    kind="ExternalOutput")[:],
            "dbg_c2c": nc.dram_tensor("dbg_c2c", [128, R, H + 2], F32,
                                      kind="ExternalOutput")[:],
            "dbg_s2c": nc.dram_tensor("dbg_s2c", [128, R, H + 2], F32,
                                      kind="ExternalOutput")[:],
            "dbg_fts": nc.dram_tensor("dbg_fts", [128, R, SW], F32,
                                      kind="ExternalOutput")[:],
            "dbg_gts": nc.dram_tensor("dbg_gts", [128, R, SW], F32,
                                      kind="ExternalOutput")[:],
        }

    with ExitStack() as ctx:
        tc = ctx.enter_context(tile.TileContext(nc))
        _emit(ctx, tc, x_d[:], wproj_d[:], smat_d[:], out_d[:], z_dtype, dbg)
    split_multiwaits(nc)
    return nc


def _emit(ctx, tc, x_d, wproj_d, smat_d, out_d, z_dtype, dbg=None):
    nc = tc.nc

    persist = ctx.enter_context(tc.tile_pool(name="persist", bufs=1))
    psum = ctx.enter_context(tc.tile_pool(name="psum", bufs=2, space="PSUM"))
    psum_sh = ctx.enter_context(tc.tile_pool(name="psum_sh", bufs=1, space="PSUM"))
    strip_pool = ctx.enter_context(tc.tile_pool(name="strip", bufs=2))
    pairtmp = ctx.enter_context(tc.tile_pool(name="pairtmp", bufs=2))
    accp = ctx.enter_context(tc.tile_pool(name="accp", bufs=4))

    # ---------------- persistent tensors ----------------
    wproj_sb = persist.tile([C + 1, NFEAT], F32)
    nc.sync.dma_start(out=wproj_sb, in_=wproj_d)
    smat = persist.tile([128, 256], F32)       # [Sp | Sm] shift matrices
    nc.sync.dma_start(out=smat, in_=smat_d)
    smat_bf = persist.tile([128, 256], BF16)   # bf16 copy for z shifts
    nc.vector.tensor_copy(smat_bf, smat)
    SHIFT = {1: smat[:, 0:128], -1: smat[:, 128:256]}
    SHIFT_BF = {1: smat_bf[:, 0:128], -1: smat_bf[:, 128:256]}

    # bias constants for ACT ops
    cb = persist.tile([128, 4], F32)
    nc.vector.memset(cb[:, 0:1], 1e-30)
    nc.vector.memset(cb[:, 1:2], 2e-4)
    nc.vector.memset(cb[:, 2:3], 1e-6)
    nc.vector.memset(cb[:, 3:4], HALF_PI)

    # raw gate/geom staging: [128, 16 fields, 130 (guarded rows)]
    gq = persist.tile([128, 16, H + 2], F32)
    nc.vector.memset(gq[:, :, 0], 0.0)
    nc.vector.memset(gq[:, :, H + 1], 0.0)

    # full-image small planes [128, R, H+2]
    eg = persist.tile([128, R, H + 2], F32)
    mu = persist.tile([128, R, H + 2], F32)
    mup = persist.tile([128, R, H + 2], F32)   # mu[g+1] (zero pad)
    mum = persist.tile([128, R, H + 2], F32)   # mu[g-1]
    c2cF = persist.tile([128, R, H + 2], F32)
    s2cF = persist.tile([128, R, H + 2], F32)
    comu = [persist.tile([128, R, H + 2], F32, name=f"comu{i}")
            for i in range(len(PAIRS))]

    # ---------------- phase 1: gate/geom projections (x streamed) ----------------
    for k in range(NSTRIP):
        q0 = k * STRIP
        xw = strip_pool.tile([C + 1, STRIP * 128], F32, tag="xwin")
        nc.sync.dma_start(out=xw, in_=x_d[:, q0 * 128:(q0 + STRIP) * 128])
        for j in range(STRIP):
            rho = q0 + j
            pt = psum.tile([128, 16], F32, tag="proj")
            nc.tensor.matmul(pt, xw[:, j * 128:(j + 1) * 128], wproj_sb[:, 0:16],
                             start=True, stop=True)
            nc.scalar.activation(gq[:, :, 1 + rho], pt, AF.Copy)

    # ---------------- phase 1.5: full-image fields ----------------
    nc.scalar.activation(eg[:, :, 1:H + 1], gq[:, 0:4, 1:H + 1], AF.Exp)
    for gcol in (0, H + 1):
        nc.vector.memset(eg[:, :, gcol], 0.0)
    zsum = persist.tile([128, H + 2], F32)
    nc.vector.tensor_tensor(zsum, eg[:, 0, :], eg[:, 1, :], op=OP.add)
    nc.vector.tensor_tensor(zsum, zsum, eg[:, 2, :], op=OP.add)
    nc.vector.tensor_tensor(zsum, zsum, eg[:, 3, :], op=OP.add)
    rz = persist.tile([128, H + 2], F32)
    nc.scalar.activation(rz, zsum, AF.Ln, bias=cb[:, 0:1])
    nc.scalar.activation(rz, rz, AF.Exp, scale=-1.0)
    for r in range(R):
        nc.vector.tensor_tensor(mu[:, r, :], eg[:, r, :], rz, op=OP.mult)

    # Sin table is only valid on [-pi, pi]; range-reduce 2*theta (+pi/2) with
    # one +-2pi correction (theta range here is within +-3.7).
    PI = float(np.pi)
    thq = gq[:, 4:8, :]
    m1 = persist.tile([128, R, H + 2], F32)
    m2 = persist.tile([128, R, H + 2], F32)
    tred = persist.tile([128, R, H + 2], F32)
    # s2cF = sin(2*(theta + pi*d)), d = [th < -pi/2] - [th > pi/2]
    nc.vector.tensor_scalar(m1, thq, -HALF_PI, None, op0=OP.is_lt)
    nc.vector.tensor_scalar(m2, thq, HALF_PI, None, op0=OP.is_gt)
    nc.vector.tensor_tensor(m1, m1, m2, op=OP.subtract)
    nc.vector.scalar_tensor_tensor(out=tred, in0=m1, scalar=PI, in1=thq,
                                   op0=OP.mult, op1=OP.add)
    nc.scalar.activation(s2cF, tred, AF.Sin, scale=2.0)
    # c2cF = sin(2*(theta + pi*dc) + pi/2), dc = [th < -3pi/4] - [th > pi/4]
    nc.vector.tensor_scalar(m1, thq, float(-0.75 * np.pi), None, op0=OP.is_lt)
    nc.vector.tensor_scalar(m2, thq, float(0.25 * np.pi), None, op0=OP.is_gt)
    nc.vector.tensor_tensor(m1, m1, m2, op=OP.subtract)
    nc.vector.scalar_tensor_tensor(out=tred, in0=m1, scalar=PI, in1=thq,
                                   op0=OP.mult, op1=OP.add)
    nc.scalar.activation(c2cF, tred, AF.Sin, bias=cb[:, 3:4], scale=2.0)

    for ip in range(len(PAIRS)):
        nc.vector.memset(comu[ip][:, :, 0], 0.0)
        nc.vector.memset(comu[ip][:, :, H + 1], 0.0)

    if dbg is not None:
        nc.sync.dma_start(out=dbg["dbg_mu"], in_=mu)
        nc.sync.dma_start(out=dbg["dbg_gq"], in_=gq)

    # mu shifted copies via PE (N=520 > 512 -> two chunks of 260)
    for sgn, dst in ((1, mup), (-1, mum)):
        for h in range(2):
            mq = psum_sh.tile([128, 2, H + 2], F32, tag="mush")
            nc.tensor.matmul(mq, SHIFT[sgn], mu[:, 2 * h:2 * h + 2, :],
                             start=True, stop=True)
            nc.scalar.activation(dst[:, 2 * h:2 * h + 2, :], mq, AF.Copy)

    # ---------------- phases 2+3 per strip ----------------
    for k in range(NSTRIP):
        q0 = k * STRIP
        w0 = min(max(q0 - 1, 0), H - ZW)   # z window start (image rows)

        def gqw(f0, f1):
            return gq[:, f0:f1, q0:q0 + SW]

        # strip field tiles [128, R, SW]; window col j = image row q0-1+j
        c2c = c2cF[:, :, q0:q0 + SW]
        s2c = s2cF[:, :, q0:q0 + SW]
        uh = strip_pool.tile([128, R, SW], F32, tag="uh")    # e^{raw_hyper}, ACT only
        Ft = strip_pool.tile([128, R, SW], F32, tag="Ft")    # 1+uh, DVE only
        Gt = strip_pool.tile([128, R, SW], F32, tag="Gt")    # e^{-softplus}, ACT only
        bt = strip_pool.tile([128, R, SW], F32, tag="bt")    # softplus(raw_base), ACT only
        nc.scalar.activation(uh, gqw(12, 16), AF.Exp)
        nc.vector.tensor_scalar_add(Ft, uh, 1.0)
        nc.scalar.activation(Gt, uh, AF.Ln, bias=1.0)
        nc.scalar.activation(Gt, Gt, AF.Exp, scale=-1.0)
        ub = strip_pool.tile([128, R, SW], F32, tag="ub")
        nc.scalar.activation(ub, gqw(8, 12), AF.Exp)
        nc.scalar.activation(bt, ub, AF.Ln, bias=1.0)

        # dx-shifted field copies via PE shift matmuls (zero-padded at edges;
        # pad values only feed taps where mu_n = 0, any finite value is fine)
        shifted = {}
        for name, t in (("c2c", c2c), ("s2c", s2c), ("uh", uh), ("Gt", Gt),
                        ("bt", bt)):
            d = {0: t}
            for sgn in (1, -1):
                ps = psum_sh.tile([128, R, SW], F32, tag="fsh")
                nc.tensor.matmul(ps, SHIFT[sgn], t, start=True, stop=True)
                st = strip_pool.tile([128, R, SW], F32, tag=f"{name}s{sgn}")
                if name == "uh":
                    # evac with +1 fused: shifted F = shifted(uh) + 1
                    nc.vector.tensor_scalar_add(st, ps, 1.0)
                else:
                    nc.scalar.activation(st, ps, AF.Copy)
                d[sgn] = st
            shifted[name] = d
        shifted["Ft"] = {0: Ft, 1: shifted["uh"][1], -1: shifted["uh"][-1]}

        # z projection window (rows w0..w0+ZW), then +-1 column shifts of it
        xwz = strip_pool.tile([C + 1, ZW * 128], F32, tag="xwin")
        nc.sync.dma_start(out=xwz, in_=x_d[:, w0 * 128:(w0 + ZW) * 128])
        z0t = strip_pool.tile([128, ZW, R * Cout], z_dtype, tag="zsh0", bufs=2)
        for j in range(ZW):
            ptz = psum.tile([128, R * Cout], F32, tag="projz")
            nc.tensor.matmul(ptz, xwz[:, j * 128:(j + 1) * 128],
                             wproj_sb[:, 16:NFEAT], start=True, stop=True)
            nc.scalar.activation(z0t[:, j, :], ptz, AF.Copy)
        zwin = {0: z0t, 1: None, -1: None}
        for sgn in (1, -1):
            zt = strip_pool.tile([128, ZW, R * Cout], z_dtype, tag=f"zsh{sgn}", bufs=1)
            for t in range(ZW // 2):
                zps = psum_sh.tile([128, 2 * R * Cout], F32, tag="zps")
                nc.tensor.matmul(zps, SHIFT_BF[sgn],
                                 z0t[:, 2 * t:2 * t + 2, :],
                                 start=True, stop=True)
                nc.scalar.activation(zt[:, 2 * t:2 * t + 2, :], zps, AF.Copy)
            zwin[sgn] = zt

        # denominator accumulator; init with center compat (= mu)
        den = strip_pool.tile([128, R, STRIP], F32, tag="den")
        nc.vector.tensor_copy(den, mu[:, :, 1 + q0:1 + q0 + STRIP])

        compat_t = {}
        comu_sh_t = {}
        for ip, (dy, dx) in enumerate(PAIRS):
            def S(name):
                return shifted[name][dx][:, :, 1 + dy:1 + dy + STRIP]

            def Ctr(t):
                return t[:, :, 1:1 + STRIP]

            c2 = pairtmp.tile([128, R, STRIP], F32, tag="c2")
            s2 = pairtmp.tile([128, R, STRIP], F32, tag="s2")
            q = pairtmp.tile([128, R, STRIP], F32, tag="q")
            t1 = pairtmp.tile([128, R, STRIP], F32, tag="t1")
            nc.vector.tensor_tensor(c2, Ctr(c2c), S("c2c"), op=OP.add)
            nc.vector.tensor_tensor(s2, Ctr(s2c), S("s2c"), op=OP.add)
            nc.vector.tensor_tensor(q, c2, c2, op=OP.mult)
            nc.vector.tensor_tensor(t1, s2, s2, op=OP.mult)
            nc.vector.tensor_tensor(q, q, t1, op=OP.add)
            rin = pairtmp.tile([128, R, STRIP], F32, tag="rin")
            nc.scalar.activation(rin, q, AF.Ln)
            nc.scalar.activation(rin, rin, AF.Exp, scale=-0.5)
            nc.vector.tensor_scalar(rin, rin, 1e6, None, op0=OP.min)
            nc.vector.tensor_tensor(c2, c2, rin, op=OP.mult)
            nc.vector.tensor_tensor(s2, s2, rin, op=OP.mult)
            E = pairtmp.tile([128, R, STRIP], F32, tag="E")
            iE = pairtmp.tile([128, R, STRIP], F32, tag="iE")
            bp = pairtmp.tile([128, R, STRIP], F32, tag="bp")
            nc.vector.tensor_tensor(E, Ctr(Ft), S("Ft"), op=OP.mult)
            nc.vector.tensor_tensor(iE, Ctr(Gt), S("Gt"), op=OP.mult)
            nc.vector.tensor_tensor(bp, Ctr(bt), S("bt"), op=OP.add)
            rbp = pairtmp.tile([128, R, STRIP], F32, tag="rbp")
            nc.scalar.activation(rbp, bp, AF.Ln, bias=cb[:, 1:2])
            nc.scalar.activation(rbp, rbp, AF.Exp, scale=-2.0)
            pu2 = pairtmp.tile([128, R, STRIP], F32, tag="pu2")
            ps2 = pairtmp.tile([128, R, STRIP], F32, tag="ps2")
            a1, a2, a3 = dx * dx, dy * dy, dx * dy
            if a3 == 0:
                hc = 0.5 * (a1 - a2)
                nc.vector.tensor_scalar(pu2, c2, hc, 0.5, op0=OP.mult, op1=OP.add)
                nc.vector.tensor_scalar(ps2, c2, -hc, 0.5, op0=OP.mult, op1=OP.add)
            else:
                nc.vector.tensor_scalar(pu2, s2, float(a3), 1.0, op0=OP.mult, op1=OP.add)
                nc.vector.tensor_scalar(ps2, s2, float(-a3), 1.0, op0=OP.mult, op1=OP.add)
            nc.vector.tensor_tensor(pu2, pu2, iE, op=OP.mult)
            nc.vector.tensor_tensor(ps2, ps2, E, op=OP.mult)
            nc.vector.tensor_tensor(pu2, pu2, ps2, op=OP.add)
            nc.vector.tensor_tensor(pu2, pu2, rbp, op=OP.mult)
            kern = pairtmp.tile([128, R, STRIP], F32, tag="kern")
            nc.scalar.activation(kern, pu2, AF.Exp, scale=-4.0)
            if dbg is not None and k == 0:
                nc.sync.dma_start(out=dbg["dbg_kern"][:, ip], in_=kern)
                nc.sync.dma_start(out=dbg["dbg_q"][:, ip], in_=q)
                nc.sync.dma_start(out=dbg["dbg_E"][:, ip], in_=E)

            # comu into full tensor; compat with shifted mu
            cm = comu[ip]
            nc.vector.tensor_tensor(cm[:, :, 1 + q0:1 + q0 + STRIP], kern,
                                    mu[:, :, 1 + q0:1 + q0 + STRIP], op=OP.mult)
            mu_sh = {0: mu, 1: mup, -1: mum}[dx]
            cp = pairtmp.tile([128, R, STRIP], F32, tag=f"cp{ip}")
            nc.vector.tensor_tensor(cp, kern,
                                    mu_sh[:, :, 1 + q0 + dy:1 + q0 + dy + STRIP],
                                    op=OP.mult)
            compat_t[ip] = cp
            nc.vector.tensor_tensor(den, den, cp, op=OP.add)
            # mirror compat = comu shifted by (-dy,-dx); window cols q0..q0+SW
            if dx != 0:
                ps = psum_sh.tile([128, R, SW], F32, tag="csh")
                nc.tensor.matmul(ps, SHIFT[-dx], cm[:, :, q0:q0 + SW],
                                 start=True, stop=True)
                cst = strip_pool.tile([128, R, SW], F32, tag=f"csh{ip}")
                nc.scalar.activation(cst, ps, AF.Copy)
                # local col for image row (rho - dy): (1 + q0 + i - dy) - q0
                mirror = cst[:, :, 1 - dy:1 - dy + STRIP]
            else:
                mirror = cm[:, :, 1 + q0 - dy:1 + q0 - dy + STRIP]
            comu_sh_t[ip] = mirror
            nc.vector.tensor_tensor(den, den, mirror, op=OP.add)

        if dbg is not None and k == 0:
            nc.sync.dma_start(out=dbg["dbg_den"], in_=den)
        rden = strip_pool.tile([128, R, STRIP], F32, tag="rden")
        nc.scalar.activation(rden, den, AF.Ln, bias=cb[:, 2:3])
        nc.scalar.activation(rden, rden, AF.Exp, scale=-1.0)

        wt = {}
        for ip, (dy, dx) in enumerate(PAIRS):
            w1 = strip_pool.tile([128, R, STRIP], F32, tag=f"w{ip}a")
            nc.vector.tensor_tensor(w1, compat_t[ip], rden, op=OP.mult)
            wt[(dy, dx)] = w1
            w2 = strip_pool.tile([128, R, STRIP], F32, tag=f"w{ip}b")
            nc.vector.tensor_tensor(w2, comu_sh_t[ip], rden, op=OP.mult)
            wt[(-dy, -dx)] = w2
        w0t = strip_pool.tile([128, R, STRIP], F32, tag="w0t")
        nc.vector.tensor_tensor(w0t, mu[:, :, 1 + q0:1 + q0 + STRIP], rden, op=OP.mult)
        wt[(0, 0)] = w0t

        if dbg is not None and k == 0:
            nc.sync.dma_start(out=dbg["dbg_c2c"], in_=c2cF)
            nc.sync.dma_start(out=dbg["dbg_s2c"], in_=s2cF)
            nc.sync.dma_start(out=dbg["dbg_fts"], in_=shifted["Ft"][1])
            nc.sync.dma_start(out=dbg["dbg_gts"], in_=shifted["Gt"][1])
            zf = strip_pool.tile([128, ZW, R * Cout], F32, tag="zf", bufs=1)
            nc.vector.tensor_copy(zf, z0t)
            nc.sync.dma_start(out=dbg["dbg_z"], in_=zf)
            for si_, tkey in enumerate(sorted(wt.keys())):
                nc.sync.dma_start(out=dbg["dbg_w"][:, si_, :, :], in_=wt[tkey])

        # -- stencil: bf16 accumulator chain (2x DVE mode); final op writes f32 --
        all_taps = [(0, 0)] + [t for t in wt if t != (0, 0)]
        for i in range(STRIP):
            rho = q0 + i
            ops = []
            for r in range(R):
                for (dy, dx) in all_taps:
                    if 0 <= rho + dy < H:
                        ops.append((r, dy, dx))
            acc = accp.tile([128, Cout], F32, tag="acc")
            for n_, (r, dy, dx) in enumerate(ops):
                src = zwin[dx][:, rho + dy - w0, r * Cout:(r + 1) * Cout]
                sc = wt[(dy, dx)][:, r, i:i + 1]
                if n_ == 0:
                    nc.vector.tensor_scalar(acc, src, sc, None, op0=OP.mult)
                else:
                    nc.vector.scalar_tensor_tensor(out=acc, in0=src, scalar=sc,
                                                   in1=acc, op0=OP.mult, op1=OP.add)
            nc.sync.dma_start(out=out_d[rho * 128:(rho + 1) * 128, :], in_=acc)


def _host_prep(inputs):
    x = np.asarray(inputs["x"], np.float32)
    gate_w = np.asarray(inputs["gate_w"], np.float32)
    gate_b = np.asarray(inputs["gate_b"], np.float32)
    value_w = np.asarray(inputs["value_w"], np.float32)
    geom_w = np.asarray(inputs["geom_w"], np.float32)
    geom_b = np.asarray(inputs["geom_b"], np.float32)
    pw_w = np.asarray(inputs["pw_w"], np.float32)
    pw_b = np.asarray(inputs["pw_b"], np.float32)

    M = pw_w.reshape(Cout, R, C).transpose(1, 0, 2) @ value_w      # [R, Cout, C]
    wproj = np.zeros((C + 1, NFEAT), np.float32)
    wproj[0:C, 0:4] = gate_w.T
    wproj[C, 0:4] = gate_b
    wproj[0:C, 4:16] = geom_w.T
    wproj[C, 4:16] = geom_b
    wproj[0:C, 16:] = M.transpose(2, 0, 1).reshape(C, R * Cout)
    wproj[C, 16:] = np.tile(pw_b, R) / R

    smat = np.zeros((128, 256), np.float32)
    for g in range(128):   # Sp[k, g] = 1 iff k = g+1 ; Sm[k, g] = 1 iff k = g-1
        if g + 1 < 128:
            smat[g + 1, g] = 1.0
        if g - 1 >= 0:
            smat[g - 1, 128 + g] = 1.0

    xpad = np.empty((B, C + 1, L), np.float32)
    xpad[:, 0:C, :] = x.reshape(B, C, L)
    xpad[:, C, :] = 1.0
    return xpad, wproj, smat


def kernel(**inputs) -> np.ndarray:
    if "nc" not in _CACHE:
        _CACHE["nc"] = build_program()
    nc = _CACHE["nc"]
    xpad, wproj, smat = _host_prep(inputs)
    in_maps = [{"x": xpad[b], "wproj": wproj, "smat": smat} for b in range(B)]
    res = run_bass_kernel_spmd(nc, in_maps, core_ids=list(range(NCORE)))
    out = np.stack([
        res.results[b]["out"].reshape(H, W, Cout).transpose(2, 0, 1)
        for b in range(B)
    ])
    return out.astype(np.float32)



# revision 19
# speedup vs baseline: 1.3151x; 1.3151x over previous
"""Trainium2 Bass kernel for nn_AZConv2d (fuzzy-rule hyperbolic-geometry message passing).

Self-contained: hardcodes shapes B=8,C=64,H=W=128,R=4,Cout=64; shards batch over 8 cores.

v2: stencil restructured for engine balance.
  - Phase A: per image row, two PSUM-accumulating bf16 matmuls with the stacked
    slab [x_hi(64); x_lo(64)] as weights: M1 (272 cols) gives w_hi*x for all
    features (+ w_hi*x_lo for the 16 gate/geom cols), M2 (16 cols) adds
    w_lo*x_hi for gate/geom -> gq is f32-accurate (theta degeneracy needs it),
    z stays bf16. Biases are NOT in the matmul (no ones row): they enter as
    per-rule [P,1] bias APs in the field ops; pw_b via a rank-1 matmul.
  - gq: [128, 16, 130] f32; z: zbuf [128, 256, 130] bf16 with zero halo cols.
  - Fields/kern/compat/den/w: as v1 (f32 DVE/ACT small ops).
  - Stencil: per 8-row block, 9 large DVE tensor_tensor multiplies (bf16 2x mode,
    weight broadcast over o via stride-0 AP): P_dx[g,dy,r,o,rho] = w'[g,dy,r,rho]*z.
    Weights pre-shifted across partitions by -dx (tiny PE matmuls) so products
    live on the z partition; the 36-term (dx,dy,r) sum runs on the Tensor engine
    as PSUM-accumulating matmuls with shift/identity matrices (C[g] = P[g+dx]).
    Row (dy) offsets fold into the product APs via the z halo columns.
  - Out-of-range taps have w == 0 exactly (mu zero-padding), so halo/edge
    garbage is annihilated; PSUM evacuated by ACT, DMA'd per block.
"""
import numpy as np
from contextlib import ExitStack

import concourse.bass as bass
import concourse.tile as tile
from concourse import mybir
from concourse.bass_utils import run_bass_kernel_spmd

F32 = mybir.dt.float32
BF16 = mybir.dt.bfloat16
AF = mybir.ActivationFunctionType
OP = mybir.AluOpType

B, C, H, W, R, Cout = 8, 64, 128, 128, 4, 64
L = H * W
NCORE = 8
NFEAT = 16 + R * Cout  # 272
STRIP = 32
NSTRIP = H // STRIP
SW = STRIP + 2          # field window rows per strip
BLK = 8                 # stencil rows per psum accumulation block
PAIRS = [(0, 1), (1, -1), (1, 0), (1, 1)]
HALF_PI = float(np.pi / 2)

_CACHE = {}


def split_multiwaits(nc):
    """This walrus accepts ONE sync wait per instruction: split extras into
    same-engine NoOps inserted just before the instruction."""
    n = 0
    for bb in nc.main_func.blocks:
        out = []
        for ins in bb.instructions:
            si = ins.sync_info
            if si is not None and len(si.on_wait) > 1:
                waits = list(si.on_wait)
                for w in waits[:-1]:
                    n += 1
                    nop = mybir.InstNoOp(name=f"WSPLIT-{n}")
                    nop.engine = ins.engine
                    nop.sync_info = mybir.SyncInfo(on_wait=[w], on_update=[])
                    out.append(nop)
                ins.sync_info = mybir.SyncInfo(on_wait=[waits[-1]],
                                               on_update=list(si.on_update))
            out.append(ins)
        bb.instructions[:] = out
    return n


def build_program(for_sim=False, debug=False):
    nc = bass.Bass()
    xf_d = nc.dram_tensor("xf", [C, L], F32, kind="ExternalInput")
    xh_d = nc.dram_tensor("xh", [C, L], BF16, kind="ExternalInput")
    w1_d = nc.dram_tensor("w1", [C, 256], BF16, kind="ExternalInput")
    w2_d = nc.dram_tensor("w2", [C, 16], F32, kind="ExternalInput")
    smat_d = nc.dram_tensor("smat", [128, 256], F32, kind="ExternalInput")
    smatbf_d = nc.dram_tensor("smatbf", [128, 384], BF16, kind="ExternalInput")
    aux_d = nc.dram_tensor("aux", [1, 640], BF16, kind="ExternalInput")
    cbias_d = nc.dram_tensor("cbias", [128, 40], F32, kind="ExternalInput")
    out_d = nc.dram_tensor("out", [L, Cout], F32, kind="ExternalOutput")
    dbg = None
    if debug:
        dbg = {
            "dbg_gq": nc.dram_tensor("dbg_gq", [128, 16, H + 2], F32,
                                     kind="ExternalOutput")[:],
            "dbg_z": nc.dram_tensor("dbg_z", [128, 256, H + 2], BF16,
                                    kind="ExternalOutput")[:],
            "dbg_wt": nc.dram_tensor("dbg_wt", [128, NSTRIP, 3, 3, R, STRIP],
                                     BF16, kind="ExternalOutput")[:],
            "dbg_mu": nc.dram_tensor("dbg_mu", [128, R, H + 2], F32,
                                     kind="ExternalOutput")[:],
            "dbg_den": nc.dram_tensor("dbg_den", [128, NSTRIP, R, STRIP], F32,
                                      kind="ExternalOutput")[:],
        }

    with ExitStack() as ctx:
        tc = ctx.enter_context(tile.TileContext(nc))
        _emit(ctx, tc, xf_d[:], xh_d[:], w1_d[:], w2_d[:], smat_d[:],
              smatbf_d[:], aux_d[:], cbias_d[:], out_d[:], dbg)
    if not for_sim:
        split_multiwaits(nc)
    return nc


def _emit(ctx, tc, xf_d, xh_d, w1_d, w2_d, smat_d, smatbf_d, aux_d, cbias_d,
          out_d, dbg=None):
    nc = tc.nc

    persist = ctx.enter_context(tc.tile_pool(name="persist", bufs=1))
    psumA = ctx.enter_context(tc.tile_pool(name="psumA", bufs=2, space="PSUM"))
    psumB = ctx.enter_context(tc.tile_pool(name="psumB", bufs=2, space="PSUM"))
    psum_sh = ctx.enter_context(tc.tile_pool(name="psum_sh", bufs=1, space="PSUM"))
    strip_pool = ctx.enter_context(tc.tile_pool(name="strip", bufs=2))
    pairtmp = ctx.enter_context(tc.tile_pool(name="pairtmp", bufs=1))

    # ---------------- persistent tensors ----------------
    w1_sb = persist.tile([C, 256], BF16)
    nc.sync.dma_start(out=w1_sb, in_=w1_d)
    w2_sb = persist.tile([C, 16], F32)
    nc.sync.dma_start(out=w2_sb, in_=w2_d)
    smat = persist.tile([128, 256], F32)       # [Sp | Sm]
    nc.sync.dma_start(out=smat, in_=smat_d)
    smat_bf = persist.tile([128, 384], BF16)   # [Sp | Sm | I]
    nc.sync.dma_start(out=smat_bf, in_=smatbf_d)
    SHIFT = {1: smat[:, 0:128], -1: smat[:, 128:256]}
    SHIFT_BF = {1: smat_bf[:, 0:128], -1: smat_bf[:, 128:256],
                0: smat_bf[:, 256:384]}
    aux_bf = persist.tile([1, 640], BF16)      # [ones(128) | pwb_row(512)]
    nc.sync.dma_start(out=aux_bf, in_=aux_d)
    cbias = persist.tile([128, 40], F32)
    nc.sync.dma_start(out=cbias, in_=cbias_d)

    def cbs(col):
        return cbias[:, col:col + 1]

    # bias constants for ACT ops
    cb = persist.tile([128, 4], F32)
    nc.vector.memset(cb[:, 0:1], 1e-30)
    nc.vector.memset(cb[:, 1:2], 2e-4)
    nc.vector.memset(cb[:, 2:3], 1e-6)
    nc.vector.memset(cb[:, 3:4], HALF_PI)

    # gq: [128, 16 fields, 130] f32; zbuf: [128, 256, 130] bf16, zero halo.
    gq = persist.tile([128, 16, H + 2], F32)
    nc.vector.memset(gq[:, :, 0], 0.0)
    nc.vector.memset(gq[:, :, H + 1], 0.0)
    zbuf = persist.tile([128, 256, H + 2], BF16)
    nc.vector.memset(zbuf[:, :, 0], 0.0)
    nc.vector.memset(zbuf[:, :, H + 1], 0.0)
    zview = zbuf.rearrange("p (r o) c -> p r o c", r=R)

    # full-image small planes [128, R, H+2]
    mu = persist.tile([128, R, H + 2], F32)
    mup = persist.tile([128, R, H + 2], BF16)  # mu[g+1] (zero pad)
    mum = persist.tile([128, R, H + 2], BF16)  # mu[g-1]
    c2cF = persist.tile([128, R, H + 2], F32)
    s2cF = persist.tile([128, R, H + 2], F32)
    comu = [persist.tile([128, R, H + 2], BF16, name=f"comu{i}")
            for i in range(len(PAIRS))]

    # ---------------- phase A: projections (x streamed, 2 matmuls per row:
    # z from bf16 x, gq from true f32 x for theta-degeneracy accuracy) ----
    with tc.tile_pool(name="phA", bufs=2) as pha:
        for k in range(NSTRIP):
            q0 = k * STRIP
            xwh = pha.tile([C, STRIP * 128], BF16, tag="xwinh")
            nc.sync.dma_start(out=xwh, in_=xh_d[:, q0 * 128:(q0 + STRIP) * 128])
            xwf = pha.tile([C, STRIP * 128], F32, tag="xwinf")
            nc.sync.dma_start(out=xwf, in_=xf_d[:, q0 * 128:(q0 + STRIP) * 128])
            for j in range(STRIP):
                rho = q0 + j
                pt = psumA.tile([128, NFEAT], F32, tag="proj")
                ptg, ptz = pt[:, 0:16], pt[:, 16:NFEAT]
                nc.tensor.matmul(ptz, xwh[:, j * 128:(j + 1) * 128], w1_sb,
                                 start=True, stop=True, skip_group_check=True)
                nc.tensor.matmul(ptg, xwf[:, j * 128:(j + 1) * 128], w2_sb,
                                 start=True, stop=True, skip_group_check=True)
                if j % 2 == 0:
                    nc.scalar.activation(zbuf[:, :, 1 + rho], ptz, AF.Copy)
                    nc.vector.tensor_copy(gq[:, :, 1 + rho], ptg)
                else:
                    nc.vector.tensor_copy(zbuf[:, :, 1 + rho], ptz)
                    nc.scalar.activation(gq[:, :, 1 + rho], ptg, AF.Copy)

    if dbg is not None:
        nc.sync.dma_start(out=dbg["dbg_gq"], in_=gq)
        nc.sync.dma_start(out=dbg["dbg_z"], in_=zbuf)

    # ---------------- phase B: full-image fields ----------------
    # cbias cols: 0:4 gate_b | 4:8 b_th | 8:12 2*b_th | 12:16 2*b_th+pi/2
    #   16:20 -pi/2-b_th | 20:24 pi/2-b_th | 24:28 -3pi/4-b_th | 28:32 pi/4-b_th
    #   32:36 b_base | 36:40 b_hyp
    PI = float(np.pi)
    with tc.tile_pool(name="phB", bufs=1) as phb:
        eg = phb.tile([128, R, H + 2], F32, tag="eg")
        for r in range(R):
            nc.scalar.activation(eg[:, r, 1:H + 1], gq[:, r, 1:H + 1], AF.Exp,
                                 bias=cbs(0 + r))
        for gcol in (0, H + 1):
            nc.vector.memset(eg[:, :, gcol], 0.0)
        zsum = phb.tile([128, H + 2], F32, tag="zsum")
        nc.vector.tensor_tensor(zsum, eg[:, 0, :], eg[:, 1, :], op=OP.add)
        nc.vector.tensor_tensor(zsum, zsum, eg[:, 2, :], op=OP.add)
        nc.vector.tensor_tensor(zsum, zsum, eg[:, 3, :], op=OP.add)
        rz = phb.tile([128, H + 2], F32, tag="rz")
        nc.scalar.activation(rz, zsum, AF.Ln, bias=cb[:, 0:1])
        nc.scalar.activation(rz, rz, AF.Exp, scale=-1.0)
        for r in range(R):
            nc.vector.tensor_tensor(mu[:, r, :], eg[:, r, :], rz, op=OP.mult)

        # Sin table is only valid on [-pi, pi]; range-reduce 2*(theta+b) with
        # one +-2pi correction (theta+b range here is within +-3.7).
        m1 = phb.tile([128, R, H + 2], F32, tag="m1")
        m2 = phb.tile([128, R, H + 2], F32, tag="m2")
        tred = phb.tile([128, R, H + 2], F32, tag="tred")
        for r in range(R):
            thr = gq[:, 4 + r, :]
            # s2cF_r = sin(2*(th+b) + 2pi*d), d = [th < -pi/2-b] - [th > pi/2-b]
            nc.vector.tensor_scalar(m1[:, r], thr, cbs(16 + r), None, op0=OP.is_lt)
            nc.vector.tensor_scalar(m2[:, r], thr, cbs(20 + r), None, op0=OP.is_gt)
            nc.vector.tensor_tensor(m1[:, r], m1[:, r], m2[:, r], op=OP.subtract)
            nc.vector.scalar_tensor_tensor(out=tred[:, r], in0=m1[:, r],
                                           scalar=PI, in1=thr,
                                           op0=OP.mult, op1=OP.add)
            nc.scalar.activation(s2cF[:, r], tred[:, r], AF.Sin, scale=2.0,
                                 bias=cbs(8 + r))
            # c2cF_r = sin(2*(th+b) + pi/2 + 2pi*dc)
            nc.vector.tensor_scalar(m1[:, r], thr, cbs(24 + r), None, op0=OP.is_lt)
            nc.vector.tensor_scalar(m2[:, r], thr, cbs(28 + r), None, op0=OP.is_gt)
            nc.vector.tensor_tensor(m1[:, r], m1[:, r], m2[:, r], op=OP.subtract)
            nc.vector.scalar_tensor_tensor(out=tred[:, r], in0=m1[:, r],
                                           scalar=PI, in1=thr,
                                           op0=OP.mult, op1=OP.add)
            nc.scalar.activation(c2cF[:, r], tred[:, r], AF.Sin, scale=2.0,
                                 bias=cbs(12 + r))

    if dbg is not None:
        nc.sync.dma_start(out=dbg["dbg_mu"], in_=mu)

    for ip in range(len(PAIRS)):
        nc.vector.memset(comu[ip], 0.0)

    # mu shifted copies via PE (N=520 > 512 -> two chunks of 260)
    for sgn, dst in ((1, mup), (-1, mum)):
        for h in range(2):
            mq = psum_sh.tile([128, 2, H + 2], F32, tag="mush")
            nc.tensor.matmul(mq, SHIFT[sgn], mu[:, 2 * h:2 * h + 2, :],
                             start=True, stop=True)
            nc.scalar.activation(dst[:, 2 * h:2 * h + 2, :], mq, AF.Copy)

    ppool = ctx.enter_context(tc.tile_pool(name="ppool", bufs=2))

    # ---------------- phase C per strip ----------------
    for k in range(NSTRIP):
        q0 = k * STRIP

        # strip field tiles [128, R, SW]; window col j = image row q0-1+j
        c2c = c2cF[:, :, q0:q0 + SW]
        s2c = s2cF[:, :, q0:q0 + SW]
        uh = strip_pool.tile([128, R, SW], F32, tag="uh")    # e^{raw_hyper+b}
        Ft = strip_pool.tile([128, R, SW], F32, tag="Ft")    # 1+uh, DVE only
        Gt = strip_pool.tile([128, R, SW], F32, tag="Gt")    # e^{-softplus}
        bt = strip_pool.tile([128, R, SW], F32, tag="bt")    # softplus(raw_base+b)
        ub = strip_pool.tile([128, R, SW], F32, tag="ub")
        for r in range(R):
            nc.scalar.activation(uh[:, r], gq[:, 12 + r, q0:q0 + SW], AF.Exp,
                                 bias=cbs(36 + r))
            nc.scalar.activation(ub[:, r], gq[:, 8 + r, q0:q0 + SW], AF.Exp,
                                 bias=cbs(32 + r))
        nc.vector.tensor_scalar_add(Ft, uh, 1.0)
        nc.scalar.activation(Gt, uh, AF.Ln, bias=1.0)
        nc.scalar.activation(Gt, Gt, AF.Exp, scale=-1.0)
        nc.scalar.activation(bt, ub, AF.Ln, bias=1.0)

        # dx-shifted field copies via PE shift matmuls (zero-padded at edges;
        # pad values only feed taps where mu_n = 0, any finite value is fine)
        shifted = {}
        for name, t in (("c2c", c2c), ("s2c", s2c), ("uh", uh), ("Gt", Gt),
                        ("bt", bt)):
            d = {0: t}
            for sgn in (1, -1):
                ps = psum_sh.tile([128, R, SW], F32, tag="fsh")
                nc.tensor.matmul(ps, SHIFT[sgn], t, start=True, stop=True)
                st = strip_pool.tile([128, R, SW], F32, tag=f"{name}s{sgn}")
                if name == "uh":
                    # evac with +1 fused: shifted F = shifted(uh) + 1
                    nc.vector.tensor_scalar_add(st, ps, 1.0)
                else:
                    nc.scalar.activation(st, ps, AF.Copy)
                d[sgn] = st
            shifted[name] = d
        shifted["Ft"] = {0: Ft, 1: shifted["uh"][1], -1: shifted["uh"][-1]}

        # denominator accumulator; init with center compat (= mu)
        den = strip_pool.tile([128, R, STRIP], F32, tag="den")
        nc.vector.tensor_copy(den, mu[:, :, 1 + q0:1 + q0 + STRIP])

        compat_t = {}
        comu_sh_t = {}
        for ip, (dy, dx) in enumerate(PAIRS):
            def S(name):
                return shifted[name][dx][:, :, 1 + dy:1 + dy + STRIP]

            def Ctr(t):
                return t[:, :, 1:1 + STRIP]

            c2 = pairtmp.tile([128, R, STRIP], F32, tag="c2")
            s2 = pairtmp.tile([128, R, STRIP], F32, tag="s2")
            q = pairtmp.tile([128, R, STRIP], F32, tag="q")
            t1 = pairtmp.tile([128, R, STRIP], F32, tag="t1")
            nc.vector.tensor_tensor(c2, Ctr(c2c), S("c2c"), op=OP.add)
            nc.vector.tensor_tensor(s2, Ctr(s2c), S("s2c"), op=OP.add)
            nc.vector.tensor_tensor(q, c2, c2, op=OP.mult)
            nc.vector.tensor_tensor(t1, s2, s2, op=OP.mult)
            nc.vector.tensor_tensor(q, q, t1, op=OP.add)
            rin = pairtmp.tile([128, R, STRIP], F32, tag="rin")
            nc.scalar.activation(rin, q, AF.Ln)
            nc.scalar.activation(rin, rin, AF.Exp, scale=-0.5)
            nc.vector.tensor_scalar(rin, rin, 1e6, None, op0=OP.min)
            nc.vector.tensor_tensor(c2, c2, rin, op=OP.mult)
            nc.vector.tensor_tensor(s2, s2, rin, op=OP.mult)
            E = pairtmp.tile([128, R, STRIP], F32, tag="E")
            iE = pairtmp.tile([128, R, STRIP], F32, tag="iE")
            bp = pairtmp.tile([128, R, STRIP], F32, tag="bp")
            nc.vector.tensor_tensor(E, Ctr(Ft), S("Ft"), op=OP.mult)
            nc.vector.tensor_tensor(iE, Ctr(Gt), S("Gt"), op=OP.mult)
            nc.vector.tensor_tensor(bp, Ctr(bt), S("bt"), op=OP.add)
            rbp = pairtmp.tile([128, R, STRIP], F32, tag="rbp")
            nc.scalar.activation(rbp, bp, AF.Ln, bias=cb[:, 1:2])
            nc.scalar.activation(rbp, rbp, AF.Exp, scale=-2.0)
            pu2 = pairtmp.tile([128, R, STRIP], F32, tag="pu2")
            ps2 = pairtmp.tile([128, R, STRIP], F32, tag="ps2")
            a1, a2, a3 = dx * dx, dy * dy, dx * dy
            if a3 == 0:
                hc = 0.5 * (a1 - a2)
                nc.vector.tensor_scalar(pu2, c2, hc, 0.5, op0=OP.mult, op1=OP.add)
                nc.vector.tensor_scalar(ps2, c2, -hc, 0.5, op0=OP.mult, op1=OP.add)
            else:
                nc.vector.tensor_scalar(pu2, s2, float(a3), 1.0, op0=OP.mult, op1=OP.add)
                nc.vector.tensor_scalar(ps2, s2, float(-a3), 1.0, op0=OP.mult, op1=OP.add)
            nc.vector.tensor_tensor(pu2, pu2, iE, op=OP.mult)
            nc.vector.tensor_tensor(ps2, ps2, E, op=OP.mult)
            nc.vector.tensor_tensor(pu2, pu2, ps2, op=OP.add)
            nc.vector.tensor_tensor(pu2, pu2, rbp, op=OP.mult)
            kern = pairtmp.tile([128, R, STRIP], F32, tag="kern")
            nc.scalar.activation(kern, pu2, AF.Exp, scale=-4.0)

            # comu into full tensor; compat with shifted mu
            cm = comu[ip]
            nc.vector.tensor_tensor(cm[:, :, 1 + q0:1 + q0 + STRIP], kern,
                                    mu[:, :, 1 + q0:1 + q0 + STRIP], op=OP.mult)
            mu_sh = {0: mu, 1: mup, -1: mum}[dx]
            cp = pairtmp.tile([128, R, STRIP], F32, tag=f"cp{ip}")
            nc.vector.tensor_tensor(cp, kern,
                                    mu_sh[:, :, 1 + q0 + dy:1 + q0 + dy + STRIP],
                                    op=OP.mult)
            compat_t[ip] = cp
            nc.vector.tensor_tensor(den, den, cp, op=OP.add)
            # mirror compat = comu shifted by (-dy,-dx); window cols q0..q0+SW
            if dx != 0:
                ps = psum_sh.tile([128, R, SW], F32, tag="fsh")
                nc.tensor.matmul(ps, SHIFT_BF[-dx], cm[:, :, q0:q0 + SW],
                                 start=True, stop=True)
                cst = strip_pool.tile([128, R, SW], F32, tag=f"csh{ip}")
                nc.scalar.activation(cst, ps, AF.Copy)
                # local col for image row (rho - dy): (1 + q0 + i - dy) - q0
                mirror = cst[:, :, 1 - dy:1 - dy + STRIP]
            else:
                mirror = cm[:, :, 1 + q0 - dy:1 + q0 - dy + STRIP]
            comu_sh_t[ip] = mirror
            nc.vector.tensor_tensor(den, den, mirror, op=OP.add)

        if dbg is not None:
            nc.sync.dma_start(out=dbg["dbg_den"][:, k], in_=den)
        rden = strip_pool.tile([128, R, STRIP], F32, tag="rden")
        nc.scalar.activation(rden, den, AF.Ln, bias=cb[:, 2:3])
        nc.scalar.activation(rden, rden, AF.Exp, scale=-1.0)

        # normalized weights, packed bf16: Wt[g, dxi, dyi, r, rho]
        Wt = strip_pool.tile([128, 3, 3, R, STRIP], BF16, tag="Wt")
        DXI = {-1: 0, 0: 1, 1: 2}
        for ip, (dy, dx) in enumerate(PAIRS):
            w1 = pairtmp.tile([128, R, STRIP], F32, tag=f"w{ip}a")
            nc.vector.tensor_tensor(w1, compat_t[ip], rden, op=OP.mult)
            nc.gpsimd.tensor_copy(Wt[:, DXI[dx], dy + 1], w1)
            w2 = pairtmp.tile([128, R, STRIP], F32, tag=f"w{ip}b")
            nc.vector.tensor_tensor(w2, comu_sh_t[ip], rden, op=OP.mult)
            nc.gpsimd.tensor_copy(Wt[:, DXI[-dx], 1 - dy], w2)
        w0t = pairtmp.tile([128, R, STRIP], F32, tag="w0t")
        nc.vector.tensor_tensor(w0t, mu[:, :, 1 + q0:1 + q0 + STRIP], rden,
                                op=OP.mult)
        nc.gpsimd.tensor_copy(Wt[:, 1, 1], w0t)
        if dbg is not None:
            nc.sync.dma_start(out=dbg["dbg_wt"][:, k], in_=Wt)

        # partition-shift weights by -dx so products live on the z partition:
        # w'[g'] = w[g'-dx]; later C[g] = P[g+dx] via SHIFT[dx] matmul.
        Wsh = {0: Wt[:, 1]}
        for dx in (1, -1):
            ps = psum_sh.tile([128, 3, R, STRIP], F32, tag="wsh")
            nc.tensor.matmul(ps, SHIFT_BF[-dx], Wt[:, DXI[dx]],
                             start=True, stop=True)
            wshs = strip_pool.tile([128, 3, R, STRIP], BF16, tag=f"wsh{dx}")
            nc.scalar.activation(wshs, ps, AF.Copy)
            Wsh[dx] = wshs

        # ---- stencil: products on DVE (bf16 2x), 36-term sum on PE in PSUM ----
        for b in range(STRIP // BLK):
            rb = q0 + b * BLK
            rs = b * BLK
            P = {}
            for dx in (-1, 0, 1):
                Pt = ppool.tile([128, 3, R, Cout, BLK], BF16, tag=f"P{dx}")
                for dyi, dy in enumerate((-1, 0, 1)):
                    wb = Wsh[dx][:, dyi, :, None, rs:rs + BLK].to_broadcast(
                        [128, R, Cout, BLK])
                    nc.vector.tensor_tensor(
                        Pt[:, dyi],
                        zview[:, :, :, 1 + rb + dy:1 + rb + dy + BLK],
                        wb, op=OP.mult)
                P[dx] = Pt

            acc = psumB.tile([128, BLK, Cout], F32, tag="acc")
            n = 0
            for dx in (-1, 0, 1):
                for dyi in range(3):
                    for r in range(R):
                        rhs = P[dx][:, dyi, r].transpose([0, 2, 1])
                        nc.tensor.matmul(acc, SHIFT_BF[dx], rhs,
                                         start=(n == 0), stop=False,
                                         skip_group_check=True)
                        n += 1
            # + pw_b via rank-1 ones matmul (aux: ones lhsT, pwb_row rhs)
            nc.tensor.matmul(acc, aux_bf[:, 0:128], aux_bf[:, 128:640],
                             start=False, stop=True, skip_group_check=True)
            stg = ppool.tile([128, BLK, Cout], F32, tag="stg")
            nc.scalar.activation(stg, acc, AF.Copy)
            dst = out_d[rb * 128:(rb + BLK) * 128, :].rearrange(
                "(rho g) o -> g rho o", g=128)
            nc.sync.dma_start(out=dst, in_=stg)


def _host_prep(inputs):
    import ml_dtypes
    x = np.asarray(inputs["x"], np.float32)
    gate_w = np.asarray(inputs["gate_w"], np.float32)
    gate_b = np.asarray(inputs["gate_b"], np.float32)
    value_w = np.asarray(inputs["value_w"], np.float32)
    geom_w = np.asarray(inputs["geom_w"], np.float32)
    geom_b = np.asarray(inputs["geom_b"], np.float32)
    pw_w = np.asarray(inputs["pw_w"], np.float32)
    pw_b = np.asarray(inputs["pw_b"], np.float32)

    M = pw_w.reshape(Cout, R, C).transpose(1, 0, 2) @ value_w      # [R, Cout, C]
    wgq = np.concatenate([gate_w.T, geom_w.T], axis=1)             # [C, 16]
    wz = M.transpose(2, 0, 1).reshape(C, R * Cout)                 # [C, 256]

    bf = ml_dtypes.bfloat16
    w1 = wz          # [C, 256] -> bf16 (z projection)
    w2 = wgq         # [C, 16] f32 (gate/geom projection)

    smat = np.zeros((128, 384), np.float32)
    for g in range(128):   # Sp[k, g] = 1 iff k = g+1 ; Sm[k, g] = 1 iff k = g-1
        if g + 1 < 128:
            smat[g + 1, g] = 1.0
        if g - 1 >= 0:
            smat[g - 1, 128 + g] = 1.0
        smat[g, 256 + g] = 1.0
    smat_f32 = smat[:, 0:256].copy()

    aux = np.zeros((1, 640), np.float32)
    aux[0, 0:128] = 1.0
    aux[0, 128:640] = np.tile(pw_b[None, :], (BLK, 1)).reshape(-1)

    b_th, b_ba, b_hy = geom_b[0:4], geom_b[4:8], geom_b[8:12]
    cb_cols = np.zeros(40, np.float32)
    cb_cols[0:4] = gate_b
    cb_cols[4:8] = b_th
    cb_cols[8:12] = 2.0 * b_th
    cb_cols[12:16] = 2.0 * b_th + np.pi / 2
    cb_cols[16:20] = -np.pi / 2 - b_th
    cb_cols[20:24] = np.pi / 2 - b_th
    cb_cols[24:28] = -0.75 * np.pi - b_th
    cb_cols[28:32] = 0.25 * np.pi - b_th
    cb_cols[32:36] = b_ba
    cb_cols[36:40] = b_hy
    cbias = np.tile(cb_cols[None, :], (128, 1))

    xf = np.ascontiguousarray(x.reshape(B, C, L))

    return {
        "xf": xf,
        "xh": xf.astype(bf),
        "w1": w1.astype(bf),
        "w2": w2.astype(np.float32),
        "smat": smat_f32,
        "smatbf": smat.astype(bf),
        "aux": aux.astype(bf),
        "cbias": cbias,
    }


def make_in_maps(inputs):
    h = _host_prep(inputs)
    return [{"xf": h["xf"][b], "xh": h["xh"][b], "w1": h["w1"], "w2": h["w2"],
             "smat": h["smat"], "smatbf": h["smatbf"], "aux": h["aux"],
             "cbias": h["cbias"]} for b in range(B)]


def kernel(**inputs) -> np.ndarray:
    if "nc" not in _CACHE:
        _CACHE["nc"] = build_program()
    nc = _CACHE["nc"]
    in_maps = make_in_maps(inputs)
    res = run_bass_kernel_spmd(nc, in_maps, core_ids=list(range(NCORE)))
    out = np.stack([
        res.results[b]["out"].reshape(H, W, Cout).transpose(2, 0, 1)
        for b in range(B)
    ])
    return out.astype(np.float32)
